# revision 1
# baseline (speedup 1.0000x reference)
"""AggregateKNN Trainium2 kernel (8-core SPMD).

Computation (reference semantics):
  ligand_ctx = sum(ligand_atom_feature, axis=0)                     # [128]
  d2[i,j]    = |y_i|^2 + |x_j|^2 - 2 y_i.x_j                        # [4096, 65536]
  knn_idx    = top_k(-d2, 16)                                       # 16-NN per ligand
  protein_ctx = mean_i( sum_k protein_atom_feature[knn_idx[i,k]] )  # [256]
  out = concat([ligand_ctx, protein_ctx])                           # [384]

Strategy (no indices / no gather anywhere):
  All d2 GEMMs are split-operand FP32R matmuls: each fp32 value is split
  hi+lo with 11 explicit mantissa bits (K=13); 12x12-bit products are
  exact in the fp32 PSUM accumulator, so the result is bitwise a
  sequential-K fp32 matmul at 4x fp32 PE speed.
  Pass 1 (ligand-sharded): per-core d2neg for its 512 ligands vs all
    protein atoms; DVE MAX8 top-8 per 1024-wide PSUM strip; MAX8 +
    MATCH_REPLACE8 merge -> 16th/17th-largest -d2; threshold = midpoint,
    split hi/lo by mantissa masking.
  Pass 2 (protein-sharded): recompute transposed with K=15 (two extra
    rows add +t_i) so u[j,i] > 0 exactly for the 16 nearest neighbors;
    ACT Sign+accum counts per protein atom.
  The passes are pipelined over the 4 ligand groups (ligand order is
  permuted so every core owns one 128-tile of each group): while DVE
  maxes group g, ACT counts group g-1, and each group's threshold
  AllGather hides under compute.  counts @ feature GEMV (FP32R split,
  exact) + ligand-feature sum, one final 384-float AllReduce.
"""

import sys

if "/opt/trn_rl_repo" not in sys.path:
    sys.path.insert(0, "/opt/trn_rl_repo")

import numpy as np

import concourse.bass as bass
import concourse.bacc as bacc
import concourse.mybir as mybir
import concourse.tile as tile
from concourse.bass_utils import run_bass_kernel_spmd

F32 = mybir.dt.float32
F32R = mybir.dt.float32r
U32 = mybir.dt.uint32
NCORES = 8
NP_TOT = 65536          # protein atoms
NL_TOT = 4096           # ligand atoms
PF = 256                # protein feature dim
LF = 128                # ligand feature dim
K = 16

NL_LOC = NL_TOT // NCORES      # 512 ligands per core (pass 1)
NP_LOC = NP_TOT // NCORES      # 8192 protein atoms per core (pass 2)
PIECE = 4096                   # protein columns per pass-1 DMA piece
NPIECE = NP_TOT // PIECE       # 16
SW = 1024                      # PSUM strip width
LTILES = NL_LOC // 128         # 4 ligand groups
NSTRIP = NP_TOT // SW          # 64 strips per ligand group
PTILES = NP_LOC // 128         # 64 protein tiles per core (pass 2)
GCOL = NCORES * 128            # 1024 ligand columns per group
KD = 13                        # split-K rows for the d2 GEMM
KT = 15                        # + threshold hi/lo rows (pass 2)
NEG_BIG = -3.0e38
MANT_MASK = 0xFFFFF000         # keep sign+exp+11 explicit mantissa bits

_CACHE = {}


def build_nc(n_iters=1, sim_1core=False):
    nc = bacc.Bacc("TRN2", target_bir_lowering=False, debug=False,
                   num_devices=1 if sim_1core else NCORES)

    lig_local = nc.dram_tensor("lig_local", [KD, NL_LOC], F32R, kind="ExternalInput")
    lig_full = nc.dram_tensor("lig_full", [KD, NL_TOT], F32R, kind="ExternalInput")
    prot_full = nc.dram_tensor("prot_full", [KD, NP_TOT], F32R, kind="ExternalInput")
    prot_shard = nc.dram_tensor("prot_shard", [KT, NP_LOC], F32R, kind="ExternalInput")
    feat_shard = nc.dram_tensor("feat_shard", [NP_LOC, PF], F32R, kind="ExternalInput")
    ligf_local = nc.dram_tensor("ligf_local", [NL_LOC, LF], F32, kind="ExternalInput")
    out = nc.dram_tensor("out", [384], F32, kind="ExternalOutput")

    rg = [list(range(NCORES))]

    with tile.TileContext(nc) as tc:
        with (
            tc.tile_pool(name="const", bufs=1) as const,
            tc.tile_pool(name="pp", bufs=3) as pp_pool,
            tc.tile_pool(name="cdp", bufs=2) as cdp,
            tc.tile_pool(name="small", bufs=2) as small,
            tc.tile_pool(name="dram", bufs=1, space="DRAM") as dram,
        ):
            for _it in range(n_iters):
                # ---- static loads ---------------------------------------
                ligL = const.tile([KD, NL_LOC], F32R)
                nc.sync.dma_start(ligL[:], lig_local[:])
                lig6 = const.tile([KT, NL_TOT], F32R)   # rows 13,14 = th, tl
                # pass-2-only data: keep it off the SP queue that feeds the
                # first pass-1 pieces (one-shot startup latency)
                nc.scalar.dma_start(lig6[0:KD, :], lig_full[:])
                protS = const.tile([KT, NP_LOC], F32R)
                featsb = const.tile([128, PTILES, PF], F32R)
                ligfsb = const.tile([128, LTILES, LF], F32)
                ones = const.tile([128, 1], F32)
                nc.vector.memset(ones[:], 1.0)

                accs = [const.tile([128, PTILES], F32, tag=f"accA{g}",
                                   name=f"accA{g}")
                        for g in range(LTILES)]
                accB = const.tile([128, PTILES], F32)
                fixc = const.tile([128, PTILES], F32)
                nc.vector.memset(accs[LTILES - 1][:], 0.0)
                nc.vector.memset(accB[:], 0.0)
                nc.vector.memset(fixc[:], 0.0)
                nc.vector.memset(
                    fixc[:].rearrange("p (a b) -> p a b", b=2)[:, :, 0:1],
                    float(GCOL // 2))
                scrA = const.tile([128, SW], F32)
                scrB = const.tile([128, SW], F32)

                ag_ins = [dram.tile([2, 128], F32, tag=f"agi{g}",
                                    name=f"agi{g}")
                          for g in range(LTILES)]
                ag_outs = [dram.tile([2 * NCORES, 128], F32,
                                     addr_space="Local" if sim_1core
                                     else "Shared", tag=f"ago{g}",
                                     name=f"ago{g}")
                           for g in range(LTILES)]
                ar_in = dram.tile([1, 384], F32)
                ar_out = dram.tile([1, 384], F32,
                                   addr_space="Local" if sim_1core else "Shared")

                cands = [None] * LTILES

                with tc.tile_pool(name="ps2", bufs=2, space="PSUM") as ps2:
                    ps1_cm = tc.tile_pool(name="ps1", bufs=2, space="PSUM")
                    ps1 = ps1_cm.__enter__()
                    def p1_strip(g, pc, h, piece):
                        psum = ps1.tile([128, SW], F32, tag="p1s")
                        for q in range(SW // 512):
                            nc.tensor.matmul(
                                psum[:, q * 512:(q + 1) * 512],
                                ligL[:, g * 128:(g + 1) * 128],
                                piece[:, h * SW + q * 512:
                                      h * SW + (q + 1) * 512],
                                start=True, stop=True,
                            )
                        s = pc * (PIECE // SW) + h
                        nc.vector.max(cands[g][:, s * 8:(s + 1) * 8], psum[:])

                    def p2_tile(g, pt, last):
                        psum = ps2.tile([128, SW], F32, tag="p2s")
                        for q in range(SW // 512):
                            nc.tensor.matmul(
                                psum[:, q * 512:(q + 1) * 512],
                                protS[:, pt * 128:(pt + 1) * 128],
                                lig6[:, g * GCOL + q * 512:
                                     g * GCOL + (q + 1) * 512],
                                start=True, stop=True,
                            )
                        if not last:
                            nc.scalar.activation(
                                scrA[:], psum[:],
                                mybir.ActivationFunctionType.Sign,
                                accum_out=accs[g][:, pt:pt + 1],
                            )
                        elif pt % 2 == 0:
                            nc.scalar.activation(
                                scrA[:], psum[:],
                                mybir.ActivationFunctionType.Sign,
                                accum_out=accs[g][:, pt:pt + 1],
                            )
                        else:
                            nc.vector.tensor_scalar(
                                scrB[:], psum[:], 0.0, None,
                                mybir.AluOpType.is_ge, mybir.AluOpType.add,
                                accum_out=accB[:, pt:pt + 1],
                            )

                    def merge_ag(g):
                        m1 = small.tile([128, 8], F32, tag="m1")
                        sc1 = small.tile([128, NSTRIP * 8], F32, tag="sc1")
                        m2 = small.tile([128, 8], F32, tag="m2")
                        sc2 = small.tile([128, NSTRIP * 8], F32, tag="sc2")
                        m3 = small.tile([128, 8], F32, tag="m3")
                        tmid = small.tile([128, 1], F32, tag="tmid")
                        th = small.tile([128, 1], F32, tag="th")
                        tl = small.tile([128, 1], F32, tag="tl")
                        nc.vector.max(m1[:], cands[g][:])
                        nc.vector.match_replace(sc1[:], m1[:], cands[g][:], NEG_BIG)
                        nc.vector.max(m2[:], sc1[:])
                        nc.vector.match_replace(sc2[:], m2[:], sc1[:], NEG_BIG)
                        nc.vector.max(m3[:], sc2[:])
                        nc.vector.tensor_tensor(
                            tmid[:], m2[:, 7:8], m3[:, 0:1], mybir.AluOpType.add)
                        nc.vector.tensor_scalar_mul(tmid[:], tmid[:], -0.5)
                        nc.vector.tensor_scalar(
                            th[:].bitcast(U32), tmid[:].bitcast(U32),
                            MANT_MASK, None, mybir.AluOpType.bitwise_and)
                        nc.vector.tensor_tensor(
                            tl[:], tmid[:], th[:], mybir.AluOpType.subtract)
                        nc.sync.dma_start(
                            ag_ins[g][0:1, :].rearrange("a b -> b a"), th[:])
                        nc.sync.dma_start(
                            ag_ins[g][1:2, :].rearrange("a b -> b a"), tl[:])
                        if sim_1core:
                            for c in range(NCORES):
                                nc.sync.dma_start(
                                    ag_outs[g][2 * c:2 * c + 2, :],
                                    ag_ins[g][:])
                        else:
                            nc.gpsimd.collective_compute(
                                "AllGather", mybir.AluOpType.bypass,
                                ins=[ag_ins[g][:].opt()],
                                outs=[ag_outs[g][:].opt()],
                                replica_groups=rg)
                        agv = ag_outs[g][:].rearrange(
                            "(c two) n -> two c n", two=2)
                        blk = slice(g * GCOL, (g + 1) * GCOL)
                        nc.sync.dma_start(
                            lig6[KD:KD + 1, blk].bitcast(F32)
                            .rearrange("a (c n) -> a c n", c=NCORES),
                            agv[0:1, :, :])
                        nc.sync.dma_start(
                            lig6[KD + 1:KD + 2, blk].bitcast(F32)
                            .rearrange("a (c n) -> a c n", c=NCORES),
                            agv[1:2, :, :])

                    # group 0 pass 1 alone
                    cands[0] = cdp.tile([128, NSTRIP * 8], F32, tag="cand",
                                        name="cand0")
                    for pc in range(NPIECE):
                        piece = pp_pool.tile([KD, PIECE], F32R, tag="piece")
                        nc.sync.dma_start(
                            piece[:], prot_full[:, pc * PIECE:(pc + 1) * PIECE])
                        for h in range(PIECE // SW):
                            p1_strip(0, pc, h, piece)
                    # bulk loads issued here so they can't head-block the
                    # first pieces' DMA queue; they only matter from pass 2 on.
                    nc.scalar.dma_start(protS[:], prot_shard[:])
                    nc.scalar.dma_start(
                        featsb[:],
                        feat_shard.ap().rearrange("(t p) f -> p t f", p=128),
                    )
                    nc.scalar.dma_start(
                        ligfsb[:],
                        ligf_local.ap().rearrange("(t p) f -> p t f", p=128),
                    )
                    merge_ag(0)

                    # steady state: pass1(g) interleaved with pass2(g-1)
                    for g in range(1, LTILES):
                        cands[g] = cdp.tile([128, NSTRIP * 8], F32,
                                            tag="cand", name=f"cand{g}")
                        pt = 0
                        for pc in range(NPIECE):
                            piece = pp_pool.tile([KD, PIECE], F32R, tag="piece")
                            nc.sync.dma_start(
                                piece[:],
                                prot_full[:, pc * PIECE:(pc + 1) * PIECE])
                            for h2 in range(PIECE // SW // 2):
                                p1_strip(g, pc, 2 * h2, piece)
                                p1_strip(g, pc, 2 * h2 + 1, piece)
                                p2_tile(g - 1, pt, last=False)
                                p2_tile(g - 1, pt + 1, last=False)
                                pt += 2
                        merge_ag(g)

                    ps1_cm.__exit__(None, None, None)

                    # final group pass 2 (ACT + DVE halves) with the
                    # count finalization + feature GEMV interleaved per
                    # 16-ptile chunk so the tail overlaps.
                    cnt = const.tile([128, PTILES], F32)
                    cntr = const.tile([128, PTILES], F32R)
                    acc_sum = const.tile([128, PTILES], F32)
                    with tc.tile_pool(name="psv", bufs=1, space="PSUM") as psv:
                        gv = psv.tile([1, PF], F32)
                        lg = psv.tile([1, LF], F32)
                        # ligand-feature sum: independent, emit first
                        for t in range(LTILES):
                            nc.tensor.matmul(
                                lg[:], ones[:], ligfsb[:, t, :],
                                start=(t == 0), stop=(t == LTILES - 1),
                            )
                        CH = 16
                        for c0 in range(0, PTILES, CH):
                            for pt in range(c0, c0 + CH):
                                p2_tile(LTILES - 1, pt, last=True)
                            cs = slice(c0, c0 + CH)
                            nc.vector.tensor_tensor(
                                acc_sum[:, cs], accs[0][:, cs], accs[1][:, cs],
                                mybir.AluOpType.add)
                            nc.vector.tensor_tensor(
                                acc_sum[:, cs], acc_sum[:, cs], accs[2][:, cs],
                                mybir.AluOpType.add)
                            nc.vector.tensor_tensor(
                                acc_sum[:, cs], acc_sum[:, cs], accs[3][:, cs],
                                mybir.AluOpType.add)
                            nc.vector.tensor_scalar(
                                cnt[:, cs], acc_sum[:, cs], 0.5,
                                float(3 * (GCOL // 2)),
                                mybir.AluOpType.mult, mybir.AluOpType.add)
                            nc.vector.tensor_tensor(
                                cnt[:, cs], cnt[:, cs], fixc[:, cs],
                                mybir.AluOpType.add)
                            nc.vector.tensor_tensor(
                                cntr[:, cs], cnt[:, cs], accB[:, cs],
                                mybir.AluOpType.add)
                        for pt in range(PTILES):
                            nc.tensor.matmul(
                                gv[:], cntr[:, pt:pt + 1], featsb[:, pt, :],
                                start=(pt == 0), stop=(pt == PTILES - 1),
                            )
                        outsb = small.tile([1, 384], F32, tag="outsb")
                        nc.vector.tensor_copy(outsb[:, 0:LF], lg[:])
                        nc.scalar.activation(
                            outsb[:, LF:LF + PF], gv[:],
                            mybir.ActivationFunctionType.Copy,
                            scale=1.0 / NL_TOT,
                        )
                        nc.sync.dma_start(ar_in[:], outsb[:])
                        if sim_1core:
                            nc.sync.dma_start(ar_out[:], ar_in[:])
                        else:
                            nc.gpsimd.collective_compute(
                                "AllReduce", mybir.AluOpType.add,
                                ins=[ar_in[:].opt()], outs=[ar_out[:].opt()],
                                replica_groups=rg)
                        outsb2 = small.tile([1, 384], F32, tag="outsb2")
                        nc.sync.dma_start(outsb2[:], ar_out[:])
                        nc.sync.dma_start(
                            out[:].rearrange("(a b) -> a b", a=1), outsb2[:])

    nc.compile()
    return nc


def _round11(x):
    """Round fp32 to 11 explicit mantissa bits (RNE) — FP32R-exact values."""
    x64 = np.asarray(x, np.float32).astype(np.float64)
    mant, ex = np.frexp(x64)
    q = np.round(mant * (1 << 12)) / (1 << 12)
    return np.ldexp(q, ex).astype(np.float32)


def _split11(x):
    hi = _round11(x)
    lo = (np.asarray(x, np.float32) - hi).astype(np.float32)
    lo_r = _round11(lo)
    return hi, lo_r


def _perm():
    """Global ligand order: position (g, c, i) <- global tile 8g+c, elem i."""
    tiles = (np.arange(LTILES)[:, None] * NCORES +
             np.arange(NCORES)[None, :]).reshape(-1)
    return (tiles[:, None] * 128 + np.arange(128)[None, :]).reshape(-1)


def make_in_maps(protein_pos, protein_atom_feature, ligand_pos,
                 ligand_atom_feature):
    pp = np.ascontiguousarray(np.asarray(protein_pos, np.float32))
    lp = np.ascontiguousarray(np.asarray(ligand_pos, np.float32))
    pf = np.ascontiguousarray(np.asarray(protein_atom_feature, np.float32))
    lf = np.ascontiguousarray(np.asarray(ligand_atom_feature, np.float32))

    x2 = (pp * pp).sum(axis=1, dtype=np.float32)
    y2 = (lp * lp).sum(axis=1, dtype=np.float32)
    one_p = np.ones(NP_TOT, np.float32)

    lig_rows, prot_rows = [], []
    for c in range(3):
        ah, al = _split11(2.0 * lp[:, c])
        bh, bl = _split11(pp[:, c])
        lig_rows += [ah, ah, al]
        prot_rows += [bh, bl, bh]
    yh, yl = _split11(-y2)
    lig_rows += [yh, yl]
    prot_rows += [one_p, one_p]
    xh, xl = _split11(x2)
    lig_rows += [-np.ones(NL_TOT, np.float32), -np.ones(NL_TOT, np.float32)]
    prot_rows += [xh, xl]

    perm = _perm()
    lig_aug = np.ascontiguousarray(np.stack(lig_rows)[:, perm])
    prot_aug = np.ascontiguousarray(np.stack(prot_rows))
    prot_aug15 = np.concatenate(
        [prot_aug, np.ones((2, NP_TOT), np.float32)], axis=0)
    pf_r = _round11(pf)
    lf_perm = lf[perm]

    in_maps = []
    for c in range(NCORES):
        cols = (np.arange(LTILES)[:, None] * GCOL + c * 128 +
                np.arange(128)[None, :]).reshape(-1)
        in_maps.append({
            "lig_local": np.ascontiguousarray(lig_aug[:, cols]),
            "lig_full": lig_aug,
            "prot_full": prot_aug,
            "prot_shard": np.ascontiguousarray(
                prot_aug15[:, c * NP_LOC:(c + 1) * NP_LOC]),
            "feat_shard": np.ascontiguousarray(
                pf_r[c * NP_LOC:(c + 1) * NP_LOC]),
            "ligf_local": np.ascontiguousarray(lf_perm[cols]),
        })
    return in_maps


def kernel(protein_pos, protein_atom_feature, ligand_pos,
           ligand_atom_feature, k, _trace=False):
    assert int(k) == K
    if "nc" not in _CACHE:
        _CACHE["nc"] = build_nc()
    nc = _CACHE["nc"]
    in_maps = make_in_maps(protein_pos, protein_atom_feature, ligand_pos,
                           ligand_atom_feature)
    res = run_bass_kernel_spmd(nc, in_maps, core_ids=list(range(NCORES)),
                               trace=_trace)
    _CACHE["last_results"] = res
    return np.asarray(res.results[0]["out"], np.float32)


if __name__ == "__main__":
    rng = np.random.default_rng(0)
    inputs = {
        "protein_pos": rng.standard_normal((NP_TOT, 3)).astype(np.float32),
        "protein_atom_feature": rng.standard_normal((NP_TOT, PF)).astype(np.float32),
        "ligand_pos": rng.standard_normal((NL_TOT, 3)).astype(np.float32),
        "ligand_atom_feature": rng.standard_normal((NL_TOT, LF)).astype(np.float32),
        "k": 16,
    }
    out = kernel(**inputs)
    print("out[:8]:", out[:8])
    print("out[128:136]:", out[128:136])

